# revision 9
# baseline (speedup 1.0000x reference)
"""Toeplitz bias kernel for trn2 (8 NeuronCores).

bias[h, j, i] = exp(w_[h] - offset[h])[2*L-2 + j - i]   with L = 2048.

Let q = reverse(exp(w_ - offset)) (length S = 2*L-1 = 4095); then
bias[h, j, i] = q[L-1 - j + i].

Device pipeline per head (default variant pbpls_r4x16; no staircase, no
chained small copies):
  1. load the packed 16 KB row [w_rev | -offset] into SBUF partition 0,
     split hi/lo across the two HWDGE queues (the hi part carries the
     -offset column, so step 2 starts as soon as 10 KB have landed);
  2. exp on ACT over [1, S] (activation time is column-bound, so one
     partition costs the same as 128) with bias = -offset, hi columns
     first so super-block 0's broadcast is unblocked ~1.5 us earlier;
  3. gpsimd partition_broadcast replicates the exp'd row's 2559-column
     window for each 512-row output super-block into its own [128, 2560]
     tile -- engine-side, no DMA/HBM traffic. Per-super-block tiles keep
     Tile's range-based dependency tracking exact, so super-block 0's
     stores start while blocks 1-3 are still broadcasting;
  4. stores read those tiles through a *diagonal* access pattern: giving
     dim0 a stride of (pitch - 4) makes partition t start 4 elements
     (16 B, line-aligned) earlier, so partition t supplies output row
     j = 512*sb + 4t + r and a [128, L] block store is one DMA:
        src[t, i] = q[(L-1-512sb-r) - 4t + i]
     Four r-phases x four super-blocks = 16 one-MB store DMAs per head,
     alternated across the two HWDGE queues (sync/SP + scalar/ACT),
     ~4 MB in flight on each.

The store phase is HBM-write-bound (~400 GB/s/core with all 8 cores
writing, ~3.2 TB/s chip-wide); everything else is off the critical path
except ~13 us of load+exp+first-broadcast. The d=4 diagonal keeps every
per-partition descriptor 16-byte aligned -- a d=1 diagonal costs ~12%
store bandwidth.

Heads are sharded 2 per core across 8 cores; the host concatenates the
per-core [2, L, L] outputs. Host-side input prep is a pure layout
transform (row reversal + packing -offset into the spare column).
"""

import numpy as np

H = 16
L = 2048
S = 2 * L - 1  # 4095
N_CORES = 8
HPC = H // N_CORES  # heads per core
P = S + 1  # tile pitch (4096)

_cached_nc = None
DEFAULT_VARIANT = "b16"


def _build_nc_b16(variant="b16"):
    """bf16-output variant: halves HBM write traffic (16 MB/core).

    Output dram tensor is bf16 [HPC, L, L]; host upconverts to f32
    (rel err ~2e-3 vs the 2e-2 gate). Structure per head:
      - load packed row [w_rev | -offset] (f32) split hi/lo across the
        two HWDGE queues;
      - exp on ACT with output cast f32->bf16, hi cols first;
      - two 1024-row super-blocks; each gets its own [128, 3072] bf16
        window tile via gpsimd partition_broadcast (window sb: q cols
        [1024-1024sb, 4094-1024sb]);
      - d=8 diagonal stores: partition t supplies row j = 1024sb+8t+r,
        src[t, i] = window[(1023-r) - 8t + i]. Stride (P2-8)*2 = 6128 B
        is a 16 B multiple, keeping per-partition descriptors aligned.
        8 r-phases x 2 super-blocks = 16 stores of 512 KB per head.
    """
    import bass_rust
    import concourse.bacc as bacc
    import concourse.mybir as mybir
    import concourse.tile as tile

    nc = bacc.Bacc("TRN2", target_bir_lowering=False)
    f32 = mybir.dt.float32
    b16 = mybir.dt.bfloat16
    win = nc.dram_tensor("win", [HPC, P], f32, kind="ExternalInput")
    out = nc.dram_tensor("out", [HPC, L, L], b16, kind="ExternalOutput")
    P2 = 3072
    W = 3071  # window width: 1023 + 2048

    with tile.TileContext(nc) as tc:
        with tc.tile_pool(name="p", bufs=1) as pool:
            k = 0

            def store(dst, sap):
                nonlocal k
                (nc.sync, nc.scalar)[k % 2].dma_start(dst, sap)
                k += 1

            for h in range(HPC):
                wt = pool.tile([1, P], f32, tag=f"wt{h}")
                # hi cols (incl. -offset col at S) on sync, lo on scalar
                nc.sync.dma_start(wt[:, 1024:P], win[h : h + 1, 1024:P])
                nc.scalar.dma_start(wt[:, 0:1024], win[h : h + 1, 0:1024])
                qrow = pool.tile([1, S], b16, tag=f"q{h}")
                for c0_, c1_ in ((1024, S), (0, 1024)):
                    nc.scalar.activation(
                        qrow[0:1, c0_:c1_],
                        wt[0:1, c0_:c1_],
                        mybir.ActivationFunctionType.Exp,
                        bias=wt[0:1, S : S + 1],
                    )
                for sb in range(2):
                    w0 = 1024 - 1024 * sb
                    tsb = pool.tile([128, P2], b16, tag=f"t{h}_{sb}")
                    nc.gpsimd.partition_broadcast(
                        tsb[:, 0:W], qrow[0:1, w0 : w0 + W]
                    )
                    for r in range(8):
                        sap = tsb[:, 0:L]
                        sap.ap = bass_rust.VecI64Pair([[P2 - 8, 128], [1, L]])
                        sap.offset = 1023 - r
                        dst = out[h, 0:128, :]
                        dst.ap = bass_rust.VecI64Pair([[8 * L, 128], [1, L]])
                        dst.offset = (h * L + 1024 * sb + r) * L
                        store(dst, sap)
    nc.compile()
    return nc


def _build_nc_b16c(variant="b16c"):
    """Pipelined refinement of b16 (see _build_nc_b16).

    Trace findings on b16: store phase runs at the ~358 GB/s HBM-per-NC
    roofline; the losses are (a) a ~13 us load->exp->broadcast chain
    before the first store and (b) exp instructions for head 1 queued
    behind store descriptor-gen ops on the scalar engine, stalling the
    last broadcast until t=37 us.

    Fixes here:
      - program order: all loads, then all exps, then broadcasts and
        stores -- no exp ever queues behind store DGE ops;
      - exp is split into 4 column chunks (hi first) so the gpsimd
        broadcast of chunk k overlaps exp of chunk k+1;
      - broadcasts are per-1024-col chunks from qrow partition 0 into
        the per-(head, super-block) window tiles (BIR requires the
        partition_broadcast source to start at partition 0).
    """
    import bass_rust
    import concourse.bacc as bacc
    import concourse.mybir as mybir
    import concourse.tile as tile

    nc = bacc.Bacc("TRN2", target_bir_lowering=False)
    f32 = mybir.dt.float32
    b16 = mybir.dt.bfloat16
    win = nc.dram_tensor("win", [HPC, P], f32, kind="ExternalInput")
    out = nc.dram_tensor("out", [HPC, L, L], b16, kind="ExternalOutput")
    P2 = 3072
    W = 3071  # window width: 1023 + 2048

    with tile.TileContext(nc) as tc:
        with tc.tile_pool(name="p", bufs=1) as pool:
            k = 0

            def store(dst, sap):
                nonlocal k
                (nc.sync, nc.scalar)[k % 2].dma_start(dst, sap)
                k += 1

            wts, qrows = [], []
            for h in range(HPC):
                wt = pool.tile([1, P], f32, tag=f"wt{h}")
                # hi cols (incl. -offset col at S) on sync, lo on scalar
                nc.sync.dma_start(wt[:, 1024:P], win[h : h + 1, 1024:P])
                nc.scalar.dma_start(wt[:, 0:1024], win[h : h + 1, 0:1024])
                wts.append(wt)
            for h in range(HPC):
                qrow = pool.tile([1, S], b16, tag=f"q{h}")
                # hi-first 1024-col chunks: broadcast of chunk k overlaps
                # exp of chunk k+1
                for c0_, c1_ in ((3072, S), (2048, 3072), (1024, 2048), (0, 1024)):
                    nc.scalar.activation(
                        qrow[0:1, c0_:c1_],
                        wts[h][0:1, c0_:c1_],
                        mybir.ActivationFunctionType.Exp,
                        bias=wts[h][0:1, S : S + 1],
                    )
                qrows.append(qrow)
            for h in range(HPC):
                for sb in range(2):
                    # window sb covers q cols [w0, w0 + W)
                    w0 = 1024 - 1024 * sb
                    tsb = pool.tile([128, P2], b16, tag=f"t{h}_{sb}")
                    # chunk order: highest q-cols first (their exp chunks
                    # complete first)
                    for lc0, lc1 in ((2048, W), (1024, 2048), (0, 1024)):
                        nc.gpsimd.partition_broadcast(
                            tsb[:, lc0:lc1],
                            qrows[h][0:1, w0 + lc0 : w0 + lc1],
                        )
                    for r in range(8):
                        sap = tsb[:, 0:L]
                        sap.ap = bass_rust.VecI64Pair([[P2 - 8, 128], [1, L]])
                        sap.offset = 1023 - r
                        dst = out[h, 0:128, :]
                        dst.ap = bass_rust.VecI64Pair([[8 * L, 128], [1, L]])
                        dst.offset = (h * L + 1024 * sb + r) * L
                        store(dst, sap)
    nc.compile()
    return nc


def _build_nc_b16s(variant="b16s"):
    """DVE stream_shuffle broadcast variant.

    One shared [128, 4096] bf16 tile per head; exp writes partition 0;
    four stream_shuffle passes per column chunk replicate partition 0 to
    the other quadrants (lane-0 select mask; pass 0 masks lane 0 with
    255 to skip the self-copy). Stores read the shared tile through the
    d=8 diagonal: store (sb, r) has src[t, i] = Q[(2047-1024sb-r)-8t+i].

    Runs the broadcast on DVE (otherwise idle) instead of gpsimd; also
    probes the InstStreamShuffle rate for a possible hybrid.
    """
    import bass_rust
    import concourse.bacc as bacc
    import concourse.mybir as mybir
    import concourse.tile as tile

    nc = bacc.Bacc("TRN2", target_bir_lowering=False)
    f32 = mybir.dt.float32
    b16 = mybir.dt.bfloat16
    win = nc.dram_tensor("win", [HPC, P], f32, kind="ExternalInput")
    out = nc.dram_tensor("out", [HPC, L, L], b16, kind="ExternalOutput")
    PQ = 4096

    with tile.TileContext(nc) as tc:
        with tc.tile_pool(name="p", bufs=1) as pool:
            k = 0

            def store(dst, sap):
                nonlocal k
                (nc.sync, nc.scalar)[k % 2].dma_start(dst, sap)
                k += 1

            wts, qs = [], []
            for h in range(HPC):
                wt = pool.tile([1, P], f32, tag=f"wt{h}")
                nc.sync.dma_start(wt[:, 1024:P], win[h : h + 1, 1024:P])
                nc.scalar.dma_start(wt[:, 0:1024], win[h : h + 1, 0:1024])
                wts.append(wt)
            for h in range(HPC):
                q = pool.tile([128, PQ], b16, tag=f"Q{h}")
                for c0_, c1_ in ((3072, S), (2048, 3072), (1024, 2048), (0, 1024)):
                    nc.scalar.activation(
                        q[0:1, c0_:c1_],
                        wts[h][0:1, c0_:c1_],
                        mybir.ActivationFunctionType.Exp,
                        bias=wts[h][0:1, S : S + 1],
                    )
                qs.append(q)
            m0 = [255] + [0] * 31  # skip lane 0 self-copy
            mb = [0] * 32
            for h in range(HPC):
                q = qs[h]
                for c0_, c1_ in ((3072, S), (2048, 3072), (1024, 2048), (0, 1024)):
                    for a in range(4):
                        nc.vector.stream_shuffle(
                            q[32 * a : 32 * a + 32, c0_:c1_],
                            q[0:32, c0_:c1_],
                            m0 if a == 0 else mb,
                        )
                for sb in range(2):
                    for r in range(8):
                        sap = q[:, 0:L]
                        sap.ap = bass_rust.VecI64Pair([[PQ - 8, 128], [1, L]])
                        sap.offset = 2047 - 1024 * sb - r
                        dst = out[h, 0:128, :]
                        dst.ap = bass_rust.VecI64Pair([[8 * L, 128], [1, L]])
                        dst.offset = (h * L + 1024 * sb + r) * L
                        store(dst, sap)
    nc.compile()
    return nc


def _build_nc(variant=DEFAULT_VARIANT):
    if variant.startswith("b16c"):
        return _build_nc_b16c(variant)
    if variant.startswith("b16s"):
        return _build_nc_b16s(variant)
    if variant.startswith("b16"):
        return _build_nc_b16(variant)
    import bass_rust
    import concourse.bacc as bacc
    import concourse.mybir as mybir
    import concourse.tile as tile

    host_exp = variant.startswith("hx")
    q3 = variant.endswith("q3")
    fused = "r4x4" in variant
    pipelined = variant.startswith(("hxp", "pbp"))

    nc = bacc.Bacc("TRN2", target_bir_lowering=False)
    f32 = mybir.dt.float32
    win = nc.dram_tensor("win", [HPC, P], f32, kind="ExternalInput")
    out = nc.dram_tensor("out", [HPC, L, L], f32, kind="ExternalOutput")

    if pipelined:
        # Per-(head, super-block) tiles so Tile's range-based dependency
        # tracking lets sb0's stores start while sb1..3 are still being
        # broadcast. Window of super-block sb: q columns
        # [1536-512sb, 4094-512sb]; store (sb, r) reads local columns
        # (511-r) - 4t + i, always inside [0, 2559).
        P2 = 2560
        warmup = "wu" in variant
        if warmup:
            scr = nc.dram_tensor("scr", [2, 128, L], f32, kind="Internal")
        with tile.TileContext(nc) as tc:
            with tc.tile_pool(name="p", bufs=1) as pool:
                k = 0
                tail3 = "t3" in variant

                def store(dst, sap, h=0):
                    nonlocal k
                    if tail3 and h == 1:
                        eng = (nc.gpsimd, nc.sync, nc.scalar)[k % 3]
                    else:
                        eng = (nc.sync, nc.scalar)[k % 2]
                    if "qb" in variant and k == 31:
                        # rebalance 17/15: scalar's ring consistently drains
                        # ~1.8 us behind sync's; give sync the last store
                        eng = nc.sync
                    eng.dma_start(dst, sap)
                    k += 1

                if warmup:
                    # 1 MB dummy store per HWDGE queue during the otherwise
                    # idle head phase, to bring HBM/DMA arbitration up to
                    # speed before the real stores arrive (~18 us in)
                    wut = pool.tile([128, L], f32, tag="wut")
                    nc.vector.memset(wut[:, :], 1.0)
                for h in range(HPC):
                    wt = pool.tile([1, P], f32, tag=f"wt{h}")
                    if "ls" in variant:
                        # hi columns (incl. the -offset column) on one queue,
                        # lo on the other: exp_hi waits only the 10 KB hi part
                        nc.sync.dma_start(
                            wt[:, 1536:P], win[h : h + 1, 1536:P]
                        )
                        nc.scalar.dma_start(
                            wt[:, 0:1536], win[h : h + 1, 0:1536]
                        )
                    else:
                        (nc.sync, nc.scalar)[h % 2].dma_start(
                            wt[:, :], win[h : h + 1, :]
                        )
                    if warmup and h == 0:
                        nc.sync.dma_start(scr[0], wut[:, :])
                        nc.scalar.dma_start(scr[1], wut[:, :])
                    if host_exp:
                        qrow = wt
                    else:
                        qrow = pool.tile([1, S], f32, tag=f"q1{h}")
                        # hi columns first: super-block 0's broadcast only
                        # needs cols [1536, S), so it starts ~1.5 us earlier
                        for c0_, c1_ in ((1536, S), (0, 1536)):
                            nc.scalar.activation(
                                qrow[0:1, c0_:c1_],
                                wt[0:1, c0_:c1_],
                                mybir.ActivationFunctionType.Exp,
                                bias=wt[0:1, S : S + 1],
                            )
                    fuse2 = "f2" in variant
                    for sb in range(4):
                        base = 1536 - 512 * sb
                        tsb = pool.tile([128, P2], f32, tag=f"t{h}_{sb}")
                        nc.gpsimd.partition_broadcast(
                            tsb[:, 0:2559], qrow[0:1, base : base + 2559]
                        )
                        if fuse2:
                            # 2 stores/super-block: r-pairs fused via a
                            # negative middle stride on the SBUF side
                            for r0 in (0, 2):
                                sap = tsb[:, 0:L]
                                sap.ap = bass_rust.VecI64Pair(
                                    [[P2 - 4, 128], [-1, 2], [1, L]]
                                )
                                sap.offset = 511 - r0
                                dst = out[h, 0:128, :]
                                dst.ap = bass_rust.VecI64Pair(
                                    [[4 * L, 128], [L, 2], [1, L]]
                                )
                                dst.offset = (h * L + 512 * sb + r0) * L
                                store(dst, sap, h)
                        else:
                            for r in range(4):
                                sap = tsb[:, 0:L]
                                sap.ap = bass_rust.VecI64Pair(
                                    [[P2 - 4, 128], [1, L]]
                                )
                                sap.offset = 511 - r
                                dst = out[h, 0:128, :]
                                dst.ap = bass_rust.VecI64Pair(
                                    [[4 * L, 128], [1, L]]
                                )
                                dst.offset = (h * L + 512 * sb + r) * L
                                store(dst, sap, h)
        nc.compile()
        return nc

    with tile.TileContext(nc) as tc:
        with tc.tile_pool(name="p", bufs=1) as pool:
            qes = []
            for h in range(HPC):
                wt = pool.tile([1, P], f32, tag=f"wt{h}")
                qe = pool.tile([128, P], f32, tag=f"qe{h}")
                qes.append(qe)
                (nc.sync, nc.scalar)[h % 2].dma_start(
                    wt[:, :], win[h : h + 1, :]
                )
                if host_exp:
                    nc.gpsimd.partition_broadcast(qe[:, 0:S], wt[0:1, 0:S])
                else:
                    q1 = pool.tile([1, S], f32, tag=f"q1{h}")
                    nc.scalar.activation(
                        q1[:, :],
                        wt[0:1, 0:S],
                        mybir.ActivationFunctionType.Exp,
                        bias=wt[0:1, S : S + 1],
                    )
                    nc.gpsimd.partition_broadcast(qe[:, 0:S], q1[0:1, :])

            k = 0

            def store(dst, sap, h):
                nonlocal k
                engs = (
                    (nc.sync, nc.scalar, nc.gpsimd)
                    if q3
                    else (nc.sync, nc.scalar)
                )
                engs[k % len(engs)].dma_start(dst, sap)
                k += 1

            for h in range(HPC):
                if fused:
                    # 4 stores/head of 4 MB: sb fused, descending DRAM stride
                    for r in range(4):
                        c0 = L - 1 - 512 * 3 - r
                        sap = qes[h][:, 0:L]
                        sap.ap = bass_rust.VecI64Pair(
                            [[P - 4, 128], [512, 4], [1, L]]
                        )
                        sap.offset = c0
                        dst = out[h, 0:128, :]
                        dst.ap = bass_rust.VecI64Pair(
                            [[4 * L, 128], [-512 * L, 4], [1, L]]
                        )
                        dst.offset = (h * L + 512 * 3 + r) * L
                        store(dst, sap, h)
                else:
                    # 16 stores/head; partition t -> row 512*sb + 4t + r
                    for sb in range(4):
                        for r in range(4):
                            c0 = L - 1 - 512 * sb - r
                            sap = qes[h][:, 0:L]
                            sap.ap = bass_rust.VecI64Pair([[P - 4, 128], [1, L]])
                            sap.offset = c0
                            dst = out[h, 0:128, :]
                            dst.ap = bass_rust.VecI64Pair([[4 * L, 128], [1, L]])
                            dst.offset = (h * L + 512 * sb + r) * L
                            store(dst, sap, h)
    nc.compile()
    return nc


def _get_nc():
    global _cached_nc
    if _cached_nc is None:
        _cached_nc = _build_nc()
    return _cached_nc


def _make_in_maps(w_, offset, host_exp=False, chunked=False):
    w_ = np.asarray(w_, dtype=np.float32)
    offset = np.asarray(offset, dtype=np.float32)
    if chunked:
        # [H, 4, 1056]: partition p holds reversed-w cols
        # [1024p, 1024p+1024) (chunk 3: 1023 + zero pad), -offset at 1024
        wrev = w_[:, ::-1]
        win = np.zeros((H, 4, 1056), dtype=np.float32)
        for p in range(4):
            c0 = 1024 * p
            c1 = min(S, c0 + 1024)
            win[:, p, 0 : c1 - c0] = wrev[:, c0:c1]
        win[:, :, 1024] = -offset[:, None]
    else:
        win = np.zeros((H, P), dtype=np.float32)
        if host_exp:
            win[:, 0:S] = np.exp(w_[:, ::-1] - offset[:, None])
        else:
            win[:, 0:S] = w_[:, ::-1]
            win[:, S] = -offset
    in_maps = []
    for c in range(N_CORES):
        sl = slice(c * HPC, (c + 1) * HPC)
        in_maps.append({"win": np.ascontiguousarray(win[sl])})
    return in_maps


def run(w_, offset, trace=False, variant=DEFAULT_VARIANT, **trace_kw):
    import concourse.bass_utils as bu
    from concourse.bass_utils import run_bass_kernel_spmd

    if trace:
        # no fish bucket in this container; keep artifacts local
        bu.upload_artifacts = lambda tmpdir: "local://" + str(tmpdir)

    if variant == DEFAULT_VARIANT:
        nc = _get_nc()
    else:
        nc = _build_nc(variant)
    in_maps = _make_in_maps(w_, offset, host_exp=variant.startswith("hx"))
    res = run_bass_kernel_spmd(
        nc, in_maps, list(range(N_CORES)), trace=trace, **trace_kw
    )
    parts = [np.asarray(r["out"]) for r in res.results]
    full = np.concatenate(parts, axis=0)  # [H, L, L]
    if full.dtype != np.float32:
        full = full.astype(np.float32)
    return full, res


def kernel(w_, offset, seq_len=None, **_ignored):
    full, _ = run(w_, offset, trace=False)
    return full



# revision 12
# speedup vs baseline: 1.1253x; 1.1253x over previous
"""Toeplitz bias kernel for trn2 (8 NeuronCores).

bias[h, j, i] = exp(w_[h] - offset[h])[2*L-2 + j - i]   with L = 2048.

Let q = reverse(exp(w_ - offset)) (length S = 2*L-1 = 4095); then
bias[h, j, i] = q[L-1 - j + i].

Device pipeline per head (default variant pbpls_r4x16; no staircase, no
chained small copies):
  1. load the packed 16 KB row [w_rev | -offset] into SBUF partition 0,
     split hi/lo across the two HWDGE queues (the hi part carries the
     -offset column, so step 2 starts as soon as 10 KB have landed);
  2. exp on ACT over [1, S] (activation time is column-bound, so one
     partition costs the same as 128) with bias = -offset, hi columns
     first so super-block 0's broadcast is unblocked ~1.5 us earlier;
  3. gpsimd partition_broadcast replicates the exp'd row's 2559-column
     window for each 512-row output super-block into its own [128, 2560]
     tile -- engine-side, no DMA/HBM traffic. Per-super-block tiles keep
     Tile's range-based dependency tracking exact, so super-block 0's
     stores start while blocks 1-3 are still broadcasting;
  4. stores read those tiles through a *diagonal* access pattern: giving
     dim0 a stride of (pitch - 4) makes partition t start 4 elements
     (16 B, line-aligned) earlier, so partition t supplies output row
     j = 512*sb + 4t + r and a [128, L] block store is one DMA:
        src[t, i] = q[(L-1-512sb-r) - 4t + i]
     Four r-phases x four super-blocks = 16 one-MB store DMAs per head,
     alternated across the two HWDGE queues (sync/SP + scalar/ACT),
     ~4 MB in flight on each.

The store phase is HBM-write-bound (~400 GB/s/core with all 8 cores
writing, ~3.2 TB/s chip-wide); everything else is off the critical path
except ~13 us of load+exp+first-broadcast. The d=4 diagonal keeps every
per-partition descriptor 16-byte aligned -- a d=1 diagonal costs ~12%
store bandwidth.

Heads are sharded 2 per core across 8 cores; the host concatenates the
per-core [2, L, L] outputs. Host-side input prep is a pure layout
transform (row reversal + packing -offset into the spare column).
"""

import numpy as np

H = 16
L = 2048
S = 2 * L - 1  # 4095
N_CORES = 8
HPC = H // N_CORES  # heads per core
P = S + 1  # tile pitch (4096)

_cached_nc = None
DEFAULT_VARIANT = "b16"


def _build_nc_b16(variant="b16"):
    """bf16-output variant: halves HBM write traffic (16 MB/core).

    Output dram tensor is bf16 [HPC, L, L]; host upconverts to f32
    (rel err ~2e-3 vs the 2e-2 gate). Structure per head:
      - load packed row [w_rev | -offset] (f32) split hi/lo across the
        two HWDGE queues;
      - exp on ACT with output cast f32->bf16, hi cols first;
      - two 1024-row super-blocks; each gets its own [128, 3072] bf16
        window tile via gpsimd partition_broadcast (window sb: q cols
        [1024-1024sb, 4094-1024sb]);
      - d=8 diagonal stores: partition t supplies row j = 1024sb+8t+r,
        src[t, i] = window[(1023-r) - 8t + i]. Stride (P2-8)*2 = 6128 B
        is a 16 B multiple, keeping per-partition descriptors aligned.
        8 r-phases x 2 super-blocks = 16 stores of 512 KB per head.
    """
    import bass_rust
    import concourse.bacc as bacc
    import concourse.mybir as mybir
    import concourse.tile as tile

    nc = bacc.Bacc("TRN2", target_bir_lowering=False)
    f32 = mybir.dt.float32
    b16 = mybir.dt.bfloat16
    win = nc.dram_tensor("win", [HPC, P], f32, kind="ExternalInput")
    out = nc.dram_tensor("out", [HPC, L, L], b16, kind="ExternalOutput")
    P2 = 3072
    W = 3071  # window width: 1023 + 2048

    with tile.TileContext(nc) as tc:
        with tc.tile_pool(name="p", bufs=1) as pool:
            k = 0

            def store(dst, sap):
                nonlocal k
                (nc.sync, nc.scalar)[k % 2].dma_start(dst, sap)
                k += 1

            for h in range(HPC):
                wt = pool.tile([1, P], f32, tag=f"wt{h}")
                # hi cols (incl. -offset col at S) on sync, lo on scalar
                nc.sync.dma_start(wt[:, 1024:P], win[h : h + 1, 1024:P])
                nc.scalar.dma_start(wt[:, 0:1024], win[h : h + 1, 0:1024])
                qrow = pool.tile([1, S], b16, tag=f"q{h}")
                for c0_, c1_ in ((1024, S), (0, 1024)):
                    nc.scalar.activation(
                        qrow[0:1, c0_:c1_],
                        wt[0:1, c0_:c1_],
                        mybir.ActivationFunctionType.Exp,
                        bias=wt[0:1, S : S + 1],
                    )
                for sb in range(2):
                    w0 = 1024 - 1024 * sb
                    tsb = pool.tile([128, P2], b16, tag=f"t{h}_{sb}")
                    nc.gpsimd.partition_broadcast(
                        tsb[:, 0:W], qrow[0:1, w0 : w0 + W]
                    )
                    for r in range(8):
                        sap = tsb[:, 0:L]
                        sap.ap = bass_rust.VecI64Pair([[P2 - 8, 128], [1, L]])
                        sap.offset = 1023 - r
                        dst = out[h, 0:128, :]
                        dst.ap = bass_rust.VecI64Pair([[8 * L, 128], [1, L]])
                        dst.offset = (h * L + 1024 * sb + r) * L
                        store(dst, sap)
    nc.compile()
    return nc


def _build_nc_b16c(variant="b16c"):
    """Pipelined refinement of b16 (see _build_nc_b16).

    Trace findings on b16: store phase runs at the ~358 GB/s HBM-per-NC
    roofline; the losses are (a) a ~13 us load->exp->broadcast chain
    before the first store and (b) exp instructions for head 1 queued
    behind store descriptor-gen ops on the scalar engine, stalling the
    last broadcast until t=37 us.

    Fixes here:
      - program order: all loads, then all exps, then broadcasts and
        stores -- no exp ever queues behind store DGE ops;
      - exp is split into 4 column chunks (hi first) so the gpsimd
        broadcast of chunk k overlaps exp of chunk k+1;
      - broadcasts are per-1024-col chunks from qrow partition 0 into
        the per-(head, super-block) window tiles (BIR requires the
        partition_broadcast source to start at partition 0).
    """
    import bass_rust
    import concourse.bacc as bacc
    import concourse.mybir as mybir
    import concourse.tile as tile

    nc = bacc.Bacc("TRN2", target_bir_lowering=False)
    f32 = mybir.dt.float32
    b16 = mybir.dt.bfloat16
    win = nc.dram_tensor("win", [HPC, P], f32, kind="ExternalInput")
    out = nc.dram_tensor("out", [HPC, L, L], b16, kind="ExternalOutput")
    P2 = 3072
    W = 3071  # window width: 1023 + 2048

    with tile.TileContext(nc) as tc:
        with tc.tile_pool(name="p", bufs=1) as pool:
            k = 0

            def store(dst, sap):
                nonlocal k
                (nc.sync, nc.scalar)[k % 2].dma_start(dst, sap)
                k += 1

            wts, qrows = [], []
            for h in range(HPC):
                wt = pool.tile([1, P], f32, tag=f"wt{h}")
                # hi cols (incl. -offset col at S) on sync, lo on scalar
                nc.sync.dma_start(wt[:, 1024:P], win[h : h + 1, 1024:P])
                nc.scalar.dma_start(wt[:, 0:1024], win[h : h + 1, 0:1024])
                wts.append(wt)
            for h in range(HPC):
                qrow = pool.tile([1, S], b16, tag=f"q{h}")
                # hi-first 1024-col chunks: broadcast of chunk k overlaps
                # exp of chunk k+1
                for c0_, c1_ in ((3072, S), (2048, 3072), (1024, 2048), (0, 1024)):
                    nc.scalar.activation(
                        qrow[0:1, c0_:c1_],
                        wts[h][0:1, c0_:c1_],
                        mybir.ActivationFunctionType.Exp,
                        bias=wts[h][0:1, S : S + 1],
                    )
                qrows.append(qrow)
            for h in range(HPC):
                for sb in range(2):
                    # window sb covers q cols [w0, w0 + W)
                    w0 = 1024 - 1024 * sb
                    tsb = pool.tile([128, P2], b16, tag=f"t{h}_{sb}")
                    # chunk order: highest q-cols first (their exp chunks
                    # complete first)
                    for lc0, lc1 in ((2048, W), (1024, 2048), (0, 1024)):
                        nc.gpsimd.partition_broadcast(
                            tsb[:, lc0:lc1],
                            qrows[h][0:1, w0 + lc0 : w0 + lc1],
                        )
                    for r in range(8):
                        sap = tsb[:, 0:L]
                        sap.ap = bass_rust.VecI64Pair([[P2 - 8, 128], [1, L]])
                        sap.offset = 1023 - r
                        dst = out[h, 0:128, :]
                        dst.ap = bass_rust.VecI64Pair([[8 * L, 128], [1, L]])
                        dst.offset = (h * L + 1024 * sb + r) * L
                        store(dst, sap)
    nc.compile()
    return nc


def _build_nc_b16p(variant="b16p"):
    """PE-assisted ramp variant.

    Trace finding on b16/b16c: the gpsimd Q7 extended-instruction
    library load (issued ~7.4 us, ready ~14.5 us) gates the first
    partition_broadcast no matter how early its data is ready, pinning
    the first store to ~20 us.

    Fix: build head 0 / super-block 0's window via the tensor engine
    (ones[1,128].T @ q_chunk[1,512] -> PSUM) + DVE tensor_copy
    (PSUM -> SBUF with bf16 cast), which needs no Q7 library. gpsimd
    handles the other three windows (whole-window broadcasts, ready
    ~14.5 us onward), by which time the store stream is already beyond
    its first super-block. exp chunk order (1024,2048) first so PE can
    start as early as possible.
    """
    import bass_rust
    import concourse.bacc as bacc
    import concourse.mybir as mybir
    import concourse.tile as tile

    nc = bacc.Bacc("TRN2", target_bir_lowering=False)
    f32 = mybir.dt.float32
    b16 = mybir.dt.bfloat16
    win = nc.dram_tensor("win", [HPC, P], f32, kind="ExternalInput")
    out = nc.dram_tensor("out", [HPC, L, L], b16, kind="ExternalOutput")
    P2 = 3072
    W = 3071  # window width: 1023 + 2048

    with tile.TileContext(nc) as tc:
        with tc.tile_pool(name="p", bufs=1) as pool, tc.psum_pool(
            name="pp", bufs=1
        ) as pp:
            k = 0

            def store(dst, sap):
                nonlocal k
                (nc.sync, nc.scalar)[k % 2].dma_start(dst, sap)
                k += 1

            ones = pool.tile([1, 128], b16, tag="ones")
            nc.vector.memset(ones[:, :], 1.0)

            wts, qrows = [], []
            for h in range(HPC):
                wt = pool.tile([1, P], f32, tag=f"wt{h}")
                nc.sync.dma_start(wt[:, 1024:P], win[h : h + 1, 1024:P])
                nc.scalar.dma_start(wt[:, 0:1024], win[h : h + 1, 0:1024])
                wts.append(wt)
            for h in range(HPC):
                qrow = pool.tile([1, S], b16, tag=f"q{h}")
                # (1024,2048) first: it gates PE's first window chunk
                for c0_, c1_ in ((1024, 2048), (2048, 3072), (3072, S), (0, 1024)):
                    nc.scalar.activation(
                        qrow[0:1, c0_:c1_],
                        wts[h][0:1, c0_:c1_],
                        mybir.ActivationFunctionType.Exp,
                        bias=wts[h][0:1, S : S + 1],
                    )
                qrows.append(qrow)

            tiles = {}
            for h in range(HPC):
                for sb in range(2):
                    tiles[(h, sb)] = pool.tile(
                        [128, P2], b16, tag=f"t{h}_{sb}", name=f"t{h}_{sb}"
                    )

            # head 0 / sb 0 window (q cols [1024, 4095)) via PE + DVE
            t00 = tiles[(0, 0)]
            for c in range(6):
                qc0 = 1024 + 512 * c
                qc1 = min(qc0 + 512, S)
                n = qc1 - qc0
                pt = pp.tile([128, 512], f32, tag=f"ps{c}")
                nc.tensor.matmul(
                    pt[0:128, 0:n],
                    ones[0:1, 0:128],
                    qrows[0][0:1, qc0:qc1],
                    start=True,
                    stop=True,
                )
                nc.vector.tensor_copy(
                    out=t00[:, 512 * c : 512 * c + n], in_=pt[0:128, 0:n]
                )
            # remaining windows via gpsimd whole-window broadcasts
            for h, sb in ((0, 1), (1, 0), (1, 1)):
                w0 = 1024 - 1024 * sb
                nc.gpsimd.partition_broadcast(
                    tiles[(h, sb)][:, 0:W], qrows[h][0:1, w0 : w0 + W]
                )
            for h in range(HPC):
                for sb in range(2):
                    tsb = tiles[(h, sb)]
                    for r in range(8):
                        sap = tsb[:, 0:L]
                        sap.ap = bass_rust.VecI64Pair([[P2 - 8, 128], [1, L]])
                        sap.offset = 1023 - r
                        dst = out[h, 0:128, :]
                        dst.ap = bass_rust.VecI64Pair([[8 * L, 128], [1, L]])
                        dst.offset = (h * L + 1024 * sb + r) * L
                        store(dst, sap)
    nc.compile()
    return nc


def _build_nc_b16s(variant="b16s"):
    """DVE stream_shuffle broadcast variant.

    One shared [128, 4096] bf16 tile per head; exp writes partition 0;
    four stream_shuffle passes per column chunk replicate partition 0 to
    the other quadrants (lane-0 select mask; pass 0 masks lane 0 with
    255 to skip the self-copy). Stores read the shared tile through the
    d=8 diagonal: store (sb, r) has src[t, i] = Q[(2047-1024sb-r)-8t+i].

    Runs the broadcast on DVE (otherwise idle) instead of gpsimd; also
    probes the InstStreamShuffle rate for a possible hybrid.
    """
    import bass_rust
    import concourse.bacc as bacc
    import concourse.mybir as mybir
    import concourse.tile as tile

    nc = bacc.Bacc("TRN2", target_bir_lowering=False)
    f32 = mybir.dt.float32
    b16 = mybir.dt.bfloat16
    win = nc.dram_tensor("win", [HPC, P], f32, kind="ExternalInput")
    out = nc.dram_tensor("out", [HPC, L, L], b16, kind="ExternalOutput")
    PQ = 4096

    with tile.TileContext(nc) as tc:
        with tc.tile_pool(name="p", bufs=1) as pool:
            k = 0

            def store(dst, sap):
                nonlocal k
                (nc.sync, nc.scalar)[k % 2].dma_start(dst, sap)
                k += 1

            wts, qs = [], []
            for h in range(HPC):
                wt = pool.tile([1, P], f32, tag=f"wt{h}")
                nc.sync.dma_start(wt[:, 1024:P], win[h : h + 1, 1024:P])
                nc.scalar.dma_start(wt[:, 0:1024], win[h : h + 1, 0:1024])
                wts.append(wt)
            for h in range(HPC):
                q = pool.tile([128, PQ], b16, tag=f"Q{h}")
                for c0_, c1_ in ((3072, S), (2048, 3072), (1024, 2048), (0, 1024)):
                    nc.scalar.activation(
                        q[0:1, c0_:c1_],
                        wts[h][0:1, c0_:c1_],
                        mybir.ActivationFunctionType.Exp,
                        bias=wts[h][0:1, S : S + 1],
                    )
                qs.append(q)
            m0 = [255] + [0] * 31  # skip lane 0 self-copy
            mb = [0] * 32
            for h in range(HPC):
                q = qs[h]
                for c0_, c1_ in ((3072, S), (2048, 3072), (1024, 2048), (0, 1024)):
                    for a in range(4):
                        nc.vector.stream_shuffle(
                            q[32 * a : 32 * a + 32, c0_:c1_],
                            q[0:32, c0_:c1_],
                            m0 if a == 0 else mb,
                        )
                for sb in range(2):
                    for r in range(8):
                        sap = q[:, 0:L]
                        sap.ap = bass_rust.VecI64Pair([[PQ - 8, 128], [1, L]])
                        sap.offset = 2047 - 1024 * sb - r
                        dst = out[h, 0:128, :]
                        dst.ap = bass_rust.VecI64Pair([[8 * L, 128], [1, L]])
                        dst.offset = (h * L + 1024 * sb + r) * L
                        store(dst, sap)
    nc.compile()
    return nc


def _build_nc(variant=DEFAULT_VARIANT):
    if variant.startswith("b16c"):
        return _build_nc_b16c(variant)
    if variant.startswith("b16p"):
        return _build_nc_b16p(variant)
    if variant.startswith("b16s"):
        return _build_nc_b16s(variant)
    if variant.startswith("b16"):
        return _build_nc_b16(variant)
    import bass_rust
    import concourse.bacc as bacc
    import concourse.mybir as mybir
    import concourse.tile as tile

    host_exp = variant.startswith("hx")
    q3 = variant.endswith("q3")
    fused = "r4x4" in variant
    pipelined = variant.startswith(("hxp", "pbp"))

    nc = bacc.Bacc("TRN2", target_bir_lowering=False)
    f32 = mybir.dt.float32
    win = nc.dram_tensor("win", [HPC, P], f32, kind="ExternalInput")
    out = nc.dram_tensor("out", [HPC, L, L], f32, kind="ExternalOutput")

    if pipelined:
        # Per-(head, super-block) tiles so Tile's range-based dependency
        # tracking lets sb0's stores start while sb1..3 are still being
        # broadcast. Window of super-block sb: q columns
        # [1536-512sb, 4094-512sb]; store (sb, r) reads local columns
        # (511-r) - 4t + i, always inside [0, 2559).
        P2 = 2560
        warmup = "wu" in variant
        if warmup:
            scr = nc.dram_tensor("scr", [2, 128, L], f32, kind="Internal")
        with tile.TileContext(nc) as tc:
            with tc.tile_pool(name="p", bufs=1) as pool:
                k = 0
                tail3 = "t3" in variant

                def store(dst, sap, h=0):
                    nonlocal k
                    if tail3 and h == 1:
                        eng = (nc.gpsimd, nc.sync, nc.scalar)[k % 3]
                    else:
                        eng = (nc.sync, nc.scalar)[k % 2]
                    if "qb" in variant and k == 31:
                        # rebalance 17/15: scalar's ring consistently drains
                        # ~1.8 us behind sync's; give sync the last store
                        eng = nc.sync
                    eng.dma_start(dst, sap)
                    k += 1

                if warmup:
                    # 1 MB dummy store per HWDGE queue during the otherwise
                    # idle head phase, to bring HBM/DMA arbitration up to
                    # speed before the real stores arrive (~18 us in)
                    wut = pool.tile([128, L], f32, tag="wut")
                    nc.vector.memset(wut[:, :], 1.0)
                for h in range(HPC):
                    wt = pool.tile([1, P], f32, tag=f"wt{h}")
                    if "ls" in variant:
                        # hi columns (incl. the -offset column) on one queue,
                        # lo on the other: exp_hi waits only the 10 KB hi part
                        nc.sync.dma_start(
                            wt[:, 1536:P], win[h : h + 1, 1536:P]
                        )
                        nc.scalar.dma_start(
                            wt[:, 0:1536], win[h : h + 1, 0:1536]
                        )
                    else:
                        (nc.sync, nc.scalar)[h % 2].dma_start(
                            wt[:, :], win[h : h + 1, :]
                        )
                    if warmup and h == 0:
                        nc.sync.dma_start(scr[0], wut[:, :])
                        nc.scalar.dma_start(scr[1], wut[:, :])
                    if host_exp:
                        qrow = wt
                    else:
                        qrow = pool.tile([1, S], f32, tag=f"q1{h}")
                        # hi columns first: super-block 0's broadcast only
                        # needs cols [1536, S), so it starts ~1.5 us earlier
                        for c0_, c1_ in ((1536, S), (0, 1536)):
                            nc.scalar.activation(
                                qrow[0:1, c0_:c1_],
                                wt[0:1, c0_:c1_],
                                mybir.ActivationFunctionType.Exp,
                                bias=wt[0:1, S : S + 1],
                            )
                    fuse2 = "f2" in variant
                    for sb in range(4):
                        base = 1536 - 512 * sb
                        tsb = pool.tile([128, P2], f32, tag=f"t{h}_{sb}")
                        nc.gpsimd.partition_broadcast(
                            tsb[:, 0:2559], qrow[0:1, base : base + 2559]
                        )
                        if fuse2:
                            # 2 stores/super-block: r-pairs fused via a
                            # negative middle stride on the SBUF side
                            for r0 in (0, 2):
                                sap = tsb[:, 0:L]
                                sap.ap = bass_rust.VecI64Pair(
                                    [[P2 - 4, 128], [-1, 2], [1, L]]
                                )
                                sap.offset = 511 - r0
                                dst = out[h, 0:128, :]
                                dst.ap = bass_rust.VecI64Pair(
                                    [[4 * L, 128], [L, 2], [1, L]]
                                )
                                dst.offset = (h * L + 512 * sb + r0) * L
                                store(dst, sap, h)
                        else:
                            for r in range(4):
                                sap = tsb[:, 0:L]
                                sap.ap = bass_rust.VecI64Pair(
                                    [[P2 - 4, 128], [1, L]]
                                )
                                sap.offset = 511 - r
                                dst = out[h, 0:128, :]
                                dst.ap = bass_rust.VecI64Pair(
                                    [[4 * L, 128], [1, L]]
                                )
                                dst.offset = (h * L + 512 * sb + r) * L
                                store(dst, sap, h)
        nc.compile()
        return nc

    with tile.TileContext(nc) as tc:
        with tc.tile_pool(name="p", bufs=1) as pool:
            qes = []
            for h in range(HPC):
                wt = pool.tile([1, P], f32, tag=f"wt{h}")
                qe = pool.tile([128, P], f32, tag=f"qe{h}")
                qes.append(qe)
                (nc.sync, nc.scalar)[h % 2].dma_start(
                    wt[:, :], win[h : h + 1, :]
                )
                if host_exp:
                    nc.gpsimd.partition_broadcast(qe[:, 0:S], wt[0:1, 0:S])
                else:
                    q1 = pool.tile([1, S], f32, tag=f"q1{h}")
                    nc.scalar.activation(
                        q1[:, :],
                        wt[0:1, 0:S],
                        mybir.ActivationFunctionType.Exp,
                        bias=wt[0:1, S : S + 1],
                    )
                    nc.gpsimd.partition_broadcast(qe[:, 0:S], q1[0:1, :])

            k = 0

            def store(dst, sap, h):
                nonlocal k
                engs = (
                    (nc.sync, nc.scalar, nc.gpsimd)
                    if q3
                    else (nc.sync, nc.scalar)
                )
                engs[k % len(engs)].dma_start(dst, sap)
                k += 1

            for h in range(HPC):
                if fused:
                    # 4 stores/head of 4 MB: sb fused, descending DRAM stride
                    for r in range(4):
                        c0 = L - 1 - 512 * 3 - r
                        sap = qes[h][:, 0:L]
                        sap.ap = bass_rust.VecI64Pair(
                            [[P - 4, 128], [512, 4], [1, L]]
                        )
                        sap.offset = c0
                        dst = out[h, 0:128, :]
                        dst.ap = bass_rust.VecI64Pair(
                            [[4 * L, 128], [-512 * L, 4], [1, L]]
                        )
                        dst.offset = (h * L + 512 * 3 + r) * L
                        store(dst, sap, h)
                else:
                    # 16 stores/head; partition t -> row 512*sb + 4t + r
                    for sb in range(4):
                        for r in range(4):
                            c0 = L - 1 - 512 * sb - r
                            sap = qes[h][:, 0:L]
                            sap.ap = bass_rust.VecI64Pair([[P - 4, 128], [1, L]])
                            sap.offset = c0
                            dst = out[h, 0:128, :]
                            dst.ap = bass_rust.VecI64Pair([[4 * L, 128], [1, L]])
                            dst.offset = (h * L + 512 * sb + r) * L
                            store(dst, sap, h)
    nc.compile()
    return nc


def _get_nc():
    global _cached_nc
    if _cached_nc is None:
        _cached_nc = _build_nc()
    return _cached_nc


def _make_in_maps(w_, offset, host_exp=False, chunked=False):
    w_ = np.asarray(w_, dtype=np.float32)
    offset = np.asarray(offset, dtype=np.float32)
    if chunked:
        # [H, 4, 1056]: partition p holds reversed-w cols
        # [1024p, 1024p+1024) (chunk 3: 1023 + zero pad), -offset at 1024
        wrev = w_[:, ::-1]
        win = np.zeros((H, 4, 1056), dtype=np.float32)
        for p in range(4):
            c0 = 1024 * p
            c1 = min(S, c0 + 1024)
            win[:, p, 0 : c1 - c0] = wrev[:, c0:c1]
        win[:, :, 1024] = -offset[:, None]
    else:
        win = np.zeros((H, P), dtype=np.float32)
        if host_exp:
            win[:, 0:S] = np.exp(w_[:, ::-1] - offset[:, None])
        else:
            win[:, 0:S] = w_[:, ::-1]
            win[:, S] = -offset
    in_maps = []
    for c in range(N_CORES):
        sl = slice(c * HPC, (c + 1) * HPC)
        in_maps.append({"win": np.ascontiguousarray(win[sl])})
    return in_maps


def run(w_, offset, trace=False, variant=DEFAULT_VARIANT, **trace_kw):
    import concourse.bass_utils as bu
    from concourse.bass_utils import run_bass_kernel_spmd

    if trace:
        # no fish bucket in this container; keep artifacts local
        bu.upload_artifacts = lambda tmpdir: "local://" + str(tmpdir)

    if variant == DEFAULT_VARIANT:
        nc = _get_nc()
    else:
        nc = _build_nc(variant)
    in_maps = _make_in_maps(w_, offset, host_exp=variant.startswith("hx"))
    res = run_bass_kernel_spmd(
        nc, in_maps, list(range(N_CORES)), trace=trace, **trace_kw
    )
    parts = [np.asarray(r["out"]) for r in res.results]
    full = np.concatenate(parts, axis=0)  # [H, L, L]
    if full.dtype != np.float32:
        full = full.astype(np.float32)
    return full, res


def kernel(w_, offset, seq_len=None, **_ignored):
    full, _ = run(w_, offset, trace=False)
    return full



# revision 16
# speedup vs baseline: 1.1337x; 1.0075x over previous
"""Toeplitz bias kernel for trn2 (8 NeuronCores).

bias[h, j, i] = exp(w_[h] - offset[h])[2*L-2 + j - i]   with L = 2048.

Let q = reverse(exp(w_ - offset)) (length S = 2*L-1 = 4095); then
bias[h, j, i] = q[L-1 - j + i].

Device pipeline per head (default variant pbpls_r4x16; no staircase, no
chained small copies):
  1. load the packed 16 KB row [w_rev | -offset] into SBUF partition 0,
     split hi/lo across the two HWDGE queues (the hi part carries the
     -offset column, so step 2 starts as soon as 10 KB have landed);
  2. exp on ACT over [1, S] (activation time is column-bound, so one
     partition costs the same as 128) with bias = -offset, hi columns
     first so super-block 0's broadcast is unblocked ~1.5 us earlier;
  3. gpsimd partition_broadcast replicates the exp'd row's 2559-column
     window for each 512-row output super-block into its own [128, 2560]
     tile -- engine-side, no DMA/HBM traffic. Per-super-block tiles keep
     Tile's range-based dependency tracking exact, so super-block 0's
     stores start while blocks 1-3 are still broadcasting;
  4. stores read those tiles through a *diagonal* access pattern: giving
     dim0 a stride of (pitch - 4) makes partition t start 4 elements
     (16 B, line-aligned) earlier, so partition t supplies output row
     j = 512*sb + 4t + r and a [128, L] block store is one DMA:
        src[t, i] = q[(L-1-512sb-r) - 4t + i]
     Four r-phases x four super-blocks = 16 one-MB store DMAs per head,
     alternated across the two HWDGE queues (sync/SP + scalar/ACT),
     ~4 MB in flight on each.

The store phase is HBM-write-bound (~400 GB/s/core with all 8 cores
writing, ~3.2 TB/s chip-wide); everything else is off the critical path
except ~13 us of load+exp+first-broadcast. The d=4 diagonal keeps every
per-partition descriptor 16-byte aligned -- a d=1 diagonal costs ~12%
store bandwidth.

Heads are sharded 2 per core across 8 cores; the host concatenates the
per-core [2, L, L] outputs. Host-side input prep is a pure layout
transform (row reversal + packing -offset into the spare column).
"""

import numpy as np

H = 16
L = 2048
S = 2 * L - 1  # 4095
N_CORES = 8
HPC = H // N_CORES  # heads per core
P = S + 1  # tile pitch (4096)

_cached_nc = None
DEFAULT_VARIANT = "b16"


def _build_nc_b16(variant="b16"):
    """bf16-output variant: halves HBM write traffic (16 MB/core).

    Output dram tensor is bf16 [HPC, L, L]; host upconverts to f32
    (rel err ~2e-3 vs the 2e-2 gate). Structure per head:
      - load packed row [w_rev | -offset] (f32) split hi/lo across the
        two HWDGE queues;
      - exp on ACT with output cast f32->bf16, hi cols first;
      - two 1024-row super-blocks; each gets its own [128, 3072] bf16
        window tile via gpsimd partition_broadcast (window sb: q cols
        [1024-1024sb, 4094-1024sb]);
      - d=8 diagonal stores: partition t supplies row j = 1024sb+8t+r,
        src[t, i] = window[(1023-r) - 8t + i]. Stride (P2-8)*2 = 6128 B
        is a 16 B multiple, keeping per-partition descriptors aligned.
        8 r-phases x 2 super-blocks = 16 stores of 512 KB per head.
    """
    import bass_rust
    import concourse.bacc as bacc
    import concourse.mybir as mybir
    import concourse.tile as tile

    nc = bacc.Bacc("TRN2", target_bir_lowering=False)
    f32 = mybir.dt.float32
    b16 = mybir.dt.bfloat16
    win = nc.dram_tensor("win", [HPC, P], f32, kind="ExternalInput")
    out = nc.dram_tensor("out", [HPC, L, L], b16, kind="ExternalOutput")
    P2 = 3072
    W = 3071  # window width: 1023 + 2048

    with tile.TileContext(nc) as tc:
        with tc.tile_pool(name="p", bufs=1) as pool:
            k = 0

            def store(dst, sap):
                nonlocal k
                (nc.sync, nc.scalar)[k % 2].dma_start(dst, sap)
                k += 1

            for h in range(HPC):
                wt = pool.tile([1, P], f32, tag=f"wt{h}")
                # hi cols (incl. -offset col at S) on sync, lo on scalar
                nc.sync.dma_start(wt[:, 1024:P], win[h : h + 1, 1024:P])
                nc.scalar.dma_start(wt[:, 0:1024], win[h : h + 1, 0:1024])
                qrow = pool.tile([1, S], b16, tag=f"q{h}")
                for c0_, c1_ in ((1024, S), (0, 1024)):
                    nc.scalar.activation(
                        qrow[0:1, c0_:c1_],
                        wt[0:1, c0_:c1_],
                        mybir.ActivationFunctionType.Exp,
                        bias=wt[0:1, S : S + 1],
                    )
                for sb in range(2):
                    w0 = 1024 - 1024 * sb
                    tsb = pool.tile([128, P2], b16, tag=f"t{h}_{sb}")
                    nc.gpsimd.partition_broadcast(
                        tsb[:, 0:W], qrow[0:1, w0 : w0 + W]
                    )
                    for r in range(8):
                        sap = tsb[:, 0:L]
                        sap.ap = bass_rust.VecI64Pair([[P2 - 8, 128], [1, L]])
                        sap.offset = 1023 - r
                        dst = out[h, 0:128, :]
                        dst.ap = bass_rust.VecI64Pair([[8 * L, 128], [1, L]])
                        dst.offset = (h * L + 1024 * sb + r) * L
                        store(dst, sap)
    nc.compile()
    return nc


def _build_nc_b16c(variant="b16c"):
    """Pipelined refinement of b16 (see _build_nc_b16).

    Trace findings on b16: store phase runs at the ~358 GB/s HBM-per-NC
    roofline; the losses are (a) a ~13 us load->exp->broadcast chain
    before the first store and (b) exp instructions for head 1 queued
    behind store descriptor-gen ops on the scalar engine, stalling the
    last broadcast until t=37 us.

    Fixes here:
      - program order: all loads, then all exps, then broadcasts and
        stores -- no exp ever queues behind store DGE ops;
      - exp is split into 4 column chunks (hi first) so the gpsimd
        broadcast of chunk k overlaps exp of chunk k+1;
      - broadcasts are per-1024-col chunks from qrow partition 0 into
        the per-(head, super-block) window tiles (BIR requires the
        partition_broadcast source to start at partition 0).
    """
    import bass_rust
    import concourse.bacc as bacc
    import concourse.mybir as mybir
    import concourse.tile as tile

    nc = bacc.Bacc("TRN2", target_bir_lowering=False)
    f32 = mybir.dt.float32
    b16 = mybir.dt.bfloat16
    win = nc.dram_tensor("win", [HPC, P], f32, kind="ExternalInput")
    out = nc.dram_tensor("out", [HPC, L, L], b16, kind="ExternalOutput")
    P2 = 3072
    W = 3071  # window width: 1023 + 2048

    with tile.TileContext(nc) as tc:
        with tc.tile_pool(name="p", bufs=1) as pool:
            k = 0

            def store(dst, sap):
                nonlocal k
                (nc.sync, nc.scalar)[k % 2].dma_start(dst, sap)
                k += 1

            wts, qrows = [], []
            for h in range(HPC):
                wt = pool.tile([1, P], f32, tag=f"wt{h}")
                # hi cols (incl. -offset col at S) on sync, lo on scalar
                nc.sync.dma_start(wt[:, 1024:P], win[h : h + 1, 1024:P])
                nc.scalar.dma_start(wt[:, 0:1024], win[h : h + 1, 0:1024])
                wts.append(wt)
            for h in range(HPC):
                qrow = pool.tile([1, S], b16, tag=f"q{h}")
                # hi-first 1024-col chunks: broadcast of chunk k overlaps
                # exp of chunk k+1
                for c0_, c1_ in ((3072, S), (2048, 3072), (1024, 2048), (0, 1024)):
                    nc.scalar.activation(
                        qrow[0:1, c0_:c1_],
                        wts[h][0:1, c0_:c1_],
                        mybir.ActivationFunctionType.Exp,
                        bias=wts[h][0:1, S : S + 1],
                    )
                qrows.append(qrow)
            for h in range(HPC):
                for sb in range(2):
                    # window sb covers q cols [w0, w0 + W)
                    w0 = 1024 - 1024 * sb
                    tsb = pool.tile([128, P2], b16, tag=f"t{h}_{sb}")
                    # chunk order: highest q-cols first (their exp chunks
                    # complete first)
                    for lc0, lc1 in ((2048, W), (1024, 2048), (0, 1024)):
                        nc.gpsimd.partition_broadcast(
                            tsb[:, lc0:lc1],
                            qrows[h][0:1, w0 + lc0 : w0 + lc1],
                        )
                    for r in range(8):
                        sap = tsb[:, 0:L]
                        sap.ap = bass_rust.VecI64Pair([[P2 - 8, 128], [1, L]])
                        sap.offset = 1023 - r
                        dst = out[h, 0:128, :]
                        dst.ap = bass_rust.VecI64Pair([[8 * L, 128], [1, L]])
                        dst.offset = (h * L + 1024 * sb + r) * L
                        store(dst, sap)
    nc.compile()
    return nc


def _build_nc_b16p(variant="b16p"):
    """PE-assisted ramp variant.

    Trace finding on b16/b16c: the gpsimd Q7 extended-instruction
    library load (issued ~7.4 us, ready ~14.5 us) gates the first
    partition_broadcast no matter how early its data is ready, pinning
    the first store to ~20 us.

    Fix: build head 0 / super-block 0's window via the tensor engine
    (ones[1,128].T @ q_chunk[1,512] -> PSUM) + DVE tensor_copy
    (PSUM -> SBUF with bf16 cast), which needs no Q7 library. gpsimd
    handles the other three windows (whole-window broadcasts, ready
    ~14.5 us onward), by which time the store stream is already beyond
    its first super-block. exp chunk order (1024,2048) first so PE can
    start as early as possible.
    """
    import bass_rust
    import concourse.bacc as bacc
    import concourse.mybir as mybir
    import concourse.tile as tile

    nc = bacc.Bacc("TRN2", target_bir_lowering=False)
    f32 = mybir.dt.float32
    b16 = mybir.dt.bfloat16
    win = nc.dram_tensor("win", [HPC, P], f32, kind="ExternalInput")
    out = nc.dram_tensor("out", [HPC, L, L], b16, kind="ExternalOutput")
    P2 = 3072
    W = 3071  # window width: 1023 + 2048

    with tile.TileContext(nc) as tc:
        with tc.tile_pool(name="p", bufs=1) as pool, tc.psum_pool(
            name="pp", bufs=1
        ) as pp:
            k = 0

            def store(dst, sap):
                nonlocal k
                (nc.sync, nc.scalar)[k % 2].dma_start(dst, sap)
                k += 1

            ones = pool.tile([1, 128], b16, tag="ones")
            nc.vector.memset(ones[:, :], 1.0)

            wts, qrows = [], []
            for h in range(HPC):
                wt = pool.tile([1, P], f32, tag=f"wt{h}")
                nc.sync.dma_start(wt[:, 1024:P], win[h : h + 1, 1024:P])
                nc.scalar.dma_start(wt[:, 0:1024], win[h : h + 1, 0:1024])
                wts.append(wt)
            for h in range(HPC):
                qrow = pool.tile([1, S], b16, tag=f"q{h}")
                # (1024,2048) first: it gates PE's first window chunk
                for c0_, c1_ in ((1024, 2048), (2048, 3072), (3072, S), (0, 1024)):
                    nc.scalar.activation(
                        qrow[0:1, c0_:c1_],
                        wts[h][0:1, c0_:c1_],
                        mybir.ActivationFunctionType.Exp,
                        bias=wts[h][0:1, S : S + 1],
                    )
                qrows.append(qrow)

            tiles = {}
            for h in range(HPC):
                for sb in range(2):
                    tiles[(h, sb)] = pool.tile(
                        [128, P2], b16, tag=f"t{h}_{sb}", name=f"t{h}_{sb}"
                    )

            # head 0 / sb 0 window (q cols [1024, 4095)) via PE + DVE
            t00 = tiles[(0, 0)]
            for c in range(6):
                qc0 = 1024 + 512 * c
                qc1 = min(qc0 + 512, S)
                n = qc1 - qc0
                pt = pp.tile([128, 512], f32, tag=f"ps{c}")
                nc.tensor.matmul(
                    pt[0:128, 0:n],
                    ones[0:1, 0:128],
                    qrows[0][0:1, qc0:qc1],
                    start=True,
                    stop=True,
                )
                nc.vector.tensor_copy(
                    out=t00[:, 512 * c : 512 * c + n], in_=pt[0:128, 0:n]
                )
            # remaining windows via gpsimd whole-window broadcasts
            for h, sb in ((0, 1), (1, 0), (1, 1)):
                w0 = 1024 - 1024 * sb
                nc.gpsimd.partition_broadcast(
                    tiles[(h, sb)][:, 0:W], qrows[h][0:1, w0 : w0 + W]
                )
            fused = "f" in variant[4:]
            for h in range(HPC):
                for sb in range(2):
                    tsb = tiles[(h, sb)]
                    if fused:
                        # row-pair fusion: partition t covers rows
                        # 1024sb + 8t + {r, r+1} -> 8 KB contiguous
                        # DRAM run per partition, 16 stores of 1 MB
                        for r in (0, 2, 4, 6):
                            sap = tsb[:, 0:L]
                            sap.ap = bass_rust.VecI64Pair(
                                [[P2 - 8, 128], [-1, 2], [1, L]]
                            )
                            sap.offset = 1023 - r
                            dst = out[h, 0:128, :]
                            dst.ap = bass_rust.VecI64Pair(
                                [[8 * L, 128], [L, 2], [1, L]]
                            )
                            dst.offset = (h * L + 1024 * sb + r) * L
                            store(dst, sap)
                    else:
                        for r in range(8):
                            sap = tsb[:, 0:L]
                            sap.ap = bass_rust.VecI64Pair(
                                [[P2 - 8, 128], [1, L]]
                            )
                            sap.offset = 1023 - r
                            dst = out[h, 0:128, :]
                            dst.ap = bass_rust.VecI64Pair(
                                [[8 * L, 128], [1, L]]
                            )
                            dst.offset = (h * L + 1024 * sb + r) * L
                            store(dst, sap)
    nc.compile()
    return nc


def _build_nc_b16q(variant="b16q"):
    """All-PE broadcast + fused-pair stores.

    Findings driving this variant (see docstrings above + traces):
      - all 16 SDMA engines are 100% busy during the drain, at ~158 ns
        per 4 KB packet (~26 GB/s/engine); larger contiguous DRAM runs
        are the only way to raise drain rate -> fuse row-pairs into one
        store via a 3-dim AP (dst [[8L,128],[L,2],[1,L]] = 8 KB
        contiguous per partition, 16 stores of 1 MB);
      - gpsimd's Q7 library gate (~14.5 us) is avoided entirely: every
        window is built by PE (ones ⊗ chunk -> PSUM) + DVE/ACT copies;
      - input packed [4, 1056] f32 per head (chunk p + -offset at col
        1024) so exp is one [3,1024] + one [1,1024] activation instead
        of 4.5 us of serial [1, N] chunks; PE reads its rhs directly
        from partition p (no partition-0 restriction, unlike
        partition_broadcast).
    """
    import bass_rust
    import concourse.bacc as bacc
    import concourse.mybir as mybir
    import concourse.tile as tile

    nc = bacc.Bacc("TRN2", target_bir_lowering=False)
    f32 = mybir.dt.float32
    b16 = mybir.dt.bfloat16
    CN = 1056
    win = nc.dram_tensor("win", [HPC, 4, CN], f32, kind="ExternalInput")
    out = nc.dram_tensor("out", [HPC, L, L], b16, kind="ExternalOutput")
    P2 = 3072
    W = 3071

    with tile.TileContext(nc) as tc:
        with tc.tile_pool(name="p", bufs=1) as pool, tc.psum_pool(
            name="pp", bufs=1
        ) as pp:
            k = 0

            def store(dst, sap):
                nonlocal k
                (nc.sync, nc.scalar)[k % 2].dma_start(dst, sap)
                k += 1

            ones = pool.tile([1, 128], b16, tag="ones")
            nc.vector.memset(ones[:, :], 1.0)

            wts, q2s = [], []
            for h in range(HPC):
                wt = pool.tile([4, CN], f32, tag=f"wt{h}")
                # partitions 1-3 (window 0's chunks) first on sync
                nc.sync.dma_start(wt[1:4, :], win[h, 1:4, :])
                nc.scalar.dma_start(wt[0:1, :], win[h, 0:1, :])
                wts.append(wt)
            for h in range(HPC):
                q2 = pool.tile([4, 1024], b16, tag=f"q{h}")
                # [1:4] first: it alone gates window (h, 0)
                for p0, p1 in ((1, 4), (0, 1)):
                    nc.scalar.activation(
                        q2[p0:p1, :],
                        wts[h][p0:p1, 0:1024],
                        mybir.ActivationFunctionType.Exp,
                        bias=wts[h][p0:p1, 1024:1025],
                    )
                q2s.append(q2)

            psums = [pp.tile([128, 512], f32, tag=f"ps{i}", name=f"ps{i}")
                     for i in range(8)]
            ci = 0
            for h in range(HPC):
                for sb in range(2):
                    w0q = 1024 - 1024 * sb
                    tsb = pool.tile(
                        [128, P2], b16, tag=f"t{h}_{sb}", name=f"t{h}_{sb}"
                    )
                    for c in range(6):
                        g0 = w0q + 512 * c
                        g1 = min(g0 + 512, S)
                        n = g1 - g0
                        p = g0 // 1024
                        l0 = g0 % 1024
                        pt = psums[ci % 8]
                        ci += 1
                        nc.tensor.matmul(
                            pt[0:128, 0:n],
                            ones[0:1, 0:128],
                            q2s[h][p : p + 1, l0 : l0 + n],
                            start=True,
                            stop=True,
                        )
                        cp_out = tsb[:, 512 * c : 512 * c + n]
                        if c % 2 == 0:
                            nc.vector.tensor_copy(out=cp_out, in_=pt[0:128, 0:n])
                        else:
                            nc.scalar.copy(cp_out, pt[0:128, 0:n])
                    # fused row-pair stores: partition t covers rows
                    # 1024sb + 8t + {r, r+1} -> 8 KB contiguous in DRAM
                    for r in (0, 2, 4, 6):
                        sap = tsb[:, 0:L]
                        sap.ap = bass_rust.VecI64Pair(
                            [[P2 - 8, 128], [-1, 2], [1, L]]
                        )
                        sap.offset = 1023 - r
                        dst = out[h, 0:128, :]
                        dst.ap = bass_rust.VecI64Pair(
                            [[8 * L, 128], [L, 2], [1, L]]
                        )
                        dst.offset = (h * L + 1024 * sb + r) * L
                        store(dst, sap)
    nc.compile()
    return nc


def _build_nc_b16s(variant="b16s"):
    """DVE stream_shuffle broadcast variant.

    One shared [128, 4096] bf16 tile per head; exp writes partition 0;
    four stream_shuffle passes per column chunk replicate partition 0 to
    the other quadrants (lane-0 select mask; pass 0 masks lane 0 with
    255 to skip the self-copy). Stores read the shared tile through the
    d=8 diagonal: store (sb, r) has src[t, i] = Q[(2047-1024sb-r)-8t+i].

    Runs the broadcast on DVE (otherwise idle) instead of gpsimd; also
    probes the InstStreamShuffle rate for a possible hybrid.
    """
    import bass_rust
    import concourse.bacc as bacc
    import concourse.mybir as mybir
    import concourse.tile as tile

    nc = bacc.Bacc("TRN2", target_bir_lowering=False)
    f32 = mybir.dt.float32
    b16 = mybir.dt.bfloat16
    win = nc.dram_tensor("win", [HPC, P], f32, kind="ExternalInput")
    out = nc.dram_tensor("out", [HPC, L, L], b16, kind="ExternalOutput")
    PQ = 4096

    with tile.TileContext(nc) as tc:
        with tc.tile_pool(name="p", bufs=1) as pool:
            k = 0

            def store(dst, sap):
                nonlocal k
                (nc.sync, nc.scalar)[k % 2].dma_start(dst, sap)
                k += 1

            wts, qs = [], []
            for h in range(HPC):
                wt = pool.tile([1, P], f32, tag=f"wt{h}")
                nc.sync.dma_start(wt[:, 1024:P], win[h : h + 1, 1024:P])
                nc.scalar.dma_start(wt[:, 0:1024], win[h : h + 1, 0:1024])
                wts.append(wt)
            for h in range(HPC):
                q = pool.tile([128, PQ], b16, tag=f"Q{h}")
                for c0_, c1_ in ((3072, S), (2048, 3072), (1024, 2048), (0, 1024)):
                    nc.scalar.activation(
                        q[0:1, c0_:c1_],
                        wts[h][0:1, c0_:c1_],
                        mybir.ActivationFunctionType.Exp,
                        bias=wts[h][0:1, S : S + 1],
                    )
                qs.append(q)
            m0 = [255] + [0] * 31  # skip lane 0 self-copy
            mb = [0] * 32
            for h in range(HPC):
                q = qs[h]
                for c0_, c1_ in ((3072, S), (2048, 3072), (1024, 2048), (0, 1024)):
                    for a in range(4):
                        nc.vector.stream_shuffle(
                            q[32 * a : 32 * a + 32, c0_:c1_],
                            q[0:32, c0_:c1_],
                            m0 if a == 0 else mb,
                        )
                for sb in range(2):
                    for r in range(8):
                        sap = q[:, 0:L]
                        sap.ap = bass_rust.VecI64Pair([[PQ - 8, 128], [1, L]])
                        sap.offset = 2047 - 1024 * sb - r
                        dst = out[h, 0:128, :]
                        dst.ap = bass_rust.VecI64Pair([[8 * L, 128], [1, L]])
                        dst.offset = (h * L + 1024 * sb + r) * L
                        store(dst, sap)
    nc.compile()
    return nc


def _build_nc(variant=DEFAULT_VARIANT):
    if variant.startswith("b16c"):
        return _build_nc_b16c(variant)
    if variant.startswith("b16p"):
        return _build_nc_b16p(variant)
    if variant.startswith("b16q"):
        return _build_nc_b16q(variant)
    if variant.startswith("b16s"):
        return _build_nc_b16s(variant)
    if variant.startswith("b16"):
        return _build_nc_b16(variant)
    import bass_rust
    import concourse.bacc as bacc
    import concourse.mybir as mybir
    import concourse.tile as tile

    host_exp = variant.startswith("hx")
    q3 = variant.endswith("q3")
    fused = "r4x4" in variant
    pipelined = variant.startswith(("hxp", "pbp"))

    nc = bacc.Bacc("TRN2", target_bir_lowering=False)
    f32 = mybir.dt.float32
    win = nc.dram_tensor("win", [HPC, P], f32, kind="ExternalInput")
    out = nc.dram_tensor("out", [HPC, L, L], f32, kind="ExternalOutput")

    if pipelined:
        # Per-(head, super-block) tiles so Tile's range-based dependency
        # tracking lets sb0's stores start while sb1..3 are still being
        # broadcast. Window of super-block sb: q columns
        # [1536-512sb, 4094-512sb]; store (sb, r) reads local columns
        # (511-r) - 4t + i, always inside [0, 2559).
        P2 = 2560
        warmup = "wu" in variant
        if warmup:
            scr = nc.dram_tensor("scr", [2, 128, L], f32, kind="Internal")
        with tile.TileContext(nc) as tc:
            with tc.tile_pool(name="p", bufs=1) as pool:
                k = 0
                tail3 = "t3" in variant

                def store(dst, sap, h=0):
                    nonlocal k
                    if tail3 and h == 1:
                        eng = (nc.gpsimd, nc.sync, nc.scalar)[k % 3]
                    else:
                        eng = (nc.sync, nc.scalar)[k % 2]
                    if "qb" in variant and k == 31:
                        # rebalance 17/15: scalar's ring consistently drains
                        # ~1.8 us behind sync's; give sync the last store
                        eng = nc.sync
                    eng.dma_start(dst, sap)
                    k += 1

                if warmup:
                    # 1 MB dummy store per HWDGE queue during the otherwise
                    # idle head phase, to bring HBM/DMA arbitration up to
                    # speed before the real stores arrive (~18 us in)
                    wut = pool.tile([128, L], f32, tag="wut")
                    nc.vector.memset(wut[:, :], 1.0)
                for h in range(HPC):
                    wt = pool.tile([1, P], f32, tag=f"wt{h}")
                    if "ls" in variant:
                        # hi columns (incl. the -offset column) on one queue,
                        # lo on the other: exp_hi waits only the 10 KB hi part
                        nc.sync.dma_start(
                            wt[:, 1536:P], win[h : h + 1, 1536:P]
                        )
                        nc.scalar.dma_start(
                            wt[:, 0:1536], win[h : h + 1, 0:1536]
                        )
                    else:
                        (nc.sync, nc.scalar)[h % 2].dma_start(
                            wt[:, :], win[h : h + 1, :]
                        )
                    if warmup and h == 0:
                        nc.sync.dma_start(scr[0], wut[:, :])
                        nc.scalar.dma_start(scr[1], wut[:, :])
                    if host_exp:
                        qrow = wt
                    else:
                        qrow = pool.tile([1, S], f32, tag=f"q1{h}")
                        # hi columns first: super-block 0's broadcast only
                        # needs cols [1536, S), so it starts ~1.5 us earlier
                        for c0_, c1_ in ((1536, S), (0, 1536)):
                            nc.scalar.activation(
                                qrow[0:1, c0_:c1_],
                                wt[0:1, c0_:c1_],
                                mybir.ActivationFunctionType.Exp,
                                bias=wt[0:1, S : S + 1],
                            )
                    fuse2 = "f2" in variant
                    for sb in range(4):
                        base = 1536 - 512 * sb
                        tsb = pool.tile([128, P2], f32, tag=f"t{h}_{sb}")
                        nc.gpsimd.partition_broadcast(
                            tsb[:, 0:2559], qrow[0:1, base : base + 2559]
                        )
                        if fuse2:
                            # 2 stores/super-block: r-pairs fused via a
                            # negative middle stride on the SBUF side
                            for r0 in (0, 2):
                                sap = tsb[:, 0:L]
                                sap.ap = bass_rust.VecI64Pair(
                                    [[P2 - 4, 128], [-1, 2], [1, L]]
                                )
                                sap.offset = 511 - r0
                                dst = out[h, 0:128, :]
                                dst.ap = bass_rust.VecI64Pair(
                                    [[4 * L, 128], [L, 2], [1, L]]
                                )
                                dst.offset = (h * L + 512 * sb + r0) * L
                                store(dst, sap, h)
                        else:
                            for r in range(4):
                                sap = tsb[:, 0:L]
                                sap.ap = bass_rust.VecI64Pair(
                                    [[P2 - 4, 128], [1, L]]
                                )
                                sap.offset = 511 - r
                                dst = out[h, 0:128, :]
                                dst.ap = bass_rust.VecI64Pair(
                                    [[4 * L, 128], [1, L]]
                                )
                                dst.offset = (h * L + 512 * sb + r) * L
                                store(dst, sap, h)
        nc.compile()
        return nc

    with tile.TileContext(nc) as tc:
        with tc.tile_pool(name="p", bufs=1) as pool:
            qes = []
            for h in range(HPC):
                wt = pool.tile([1, P], f32, tag=f"wt{h}")
                qe = pool.tile([128, P], f32, tag=f"qe{h}")
                qes.append(qe)
                (nc.sync, nc.scalar)[h % 2].dma_start(
                    wt[:, :], win[h : h + 1, :]
                )
                if host_exp:
                    nc.gpsimd.partition_broadcast(qe[:, 0:S], wt[0:1, 0:S])
                else:
                    q1 = pool.tile([1, S], f32, tag=f"q1{h}")
                    nc.scalar.activation(
                        q1[:, :],
                        wt[0:1, 0:S],
                        mybir.ActivationFunctionType.Exp,
                        bias=wt[0:1, S : S + 1],
                    )
                    nc.gpsimd.partition_broadcast(qe[:, 0:S], q1[0:1, :])

            k = 0

            def store(dst, sap, h):
                nonlocal k
                engs = (
                    (nc.sync, nc.scalar, nc.gpsimd)
                    if q3
                    else (nc.sync, nc.scalar)
                )
                engs[k % len(engs)].dma_start(dst, sap)
                k += 1

            for h in range(HPC):
                if fused:
                    # 4 stores/head of 4 MB: sb fused, descending DRAM stride
                    for r in range(4):
                        c0 = L - 1 - 512 * 3 - r
                        sap = qes[h][:, 0:L]
                        sap.ap = bass_rust.VecI64Pair(
                            [[P - 4, 128], [512, 4], [1, L]]
                        )
                        sap.offset = c0
                        dst = out[h, 0:128, :]
                        dst.ap = bass_rust.VecI64Pair(
                            [[4 * L, 128], [-512 * L, 4], [1, L]]
                        )
                        dst.offset = (h * L + 512 * 3 + r) * L
                        store(dst, sap, h)
                else:
                    # 16 stores/head; partition t -> row 512*sb + 4t + r
                    for sb in range(4):
                        for r in range(4):
                            c0 = L - 1 - 512 * sb - r
                            sap = qes[h][:, 0:L]
                            sap.ap = bass_rust.VecI64Pair([[P - 4, 128], [1, L]])
                            sap.offset = c0
                            dst = out[h, 0:128, :]
                            dst.ap = bass_rust.VecI64Pair([[4 * L, 128], [1, L]])
                            dst.offset = (h * L + 512 * sb + r) * L
                            store(dst, sap, h)
    nc.compile()
    return nc


def _get_nc():
    global _cached_nc
    if _cached_nc is None:
        _cached_nc = _build_nc()
    return _cached_nc


def _make_in_maps(w_, offset, host_exp=False, chunked=False):
    w_ = np.asarray(w_, dtype=np.float32)
    offset = np.asarray(offset, dtype=np.float32)
    if chunked:
        # [H, 4, 1056]: partition p holds reversed-w cols
        # [1024p, 1024p+1024) (chunk 3: 1023 + zero pad), -offset at 1024
        wrev = w_[:, ::-1]
        win = np.zeros((H, 4, 1056), dtype=np.float32)
        for p in range(4):
            c0 = 1024 * p
            c1 = min(S, c0 + 1024)
            win[:, p, 0 : c1 - c0] = wrev[:, c0:c1]
        win[:, :, 1024] = -offset[:, None]
    else:
        win = np.zeros((H, P), dtype=np.float32)
        if host_exp:
            win[:, 0:S] = np.exp(w_[:, ::-1] - offset[:, None])
        else:
            win[:, 0:S] = w_[:, ::-1]
            win[:, S] = -offset
    in_maps = []
    for c in range(N_CORES):
        sl = slice(c * HPC, (c + 1) * HPC)
        in_maps.append({"win": np.ascontiguousarray(win[sl])})
    return in_maps


def run(w_, offset, trace=False, variant=DEFAULT_VARIANT, **trace_kw):
    import concourse.bass_utils as bu
    from concourse.bass_utils import run_bass_kernel_spmd

    if trace:
        # no fish bucket in this container; keep artifacts local
        bu.upload_artifacts = lambda tmpdir: "local://" + str(tmpdir)

    if variant == DEFAULT_VARIANT:
        nc = _get_nc()
    else:
        nc = _build_nc(variant)
    in_maps = _make_in_maps(
        w_,
        offset,
        host_exp=variant.startswith("hx"),
        chunked=variant.startswith("b16q"),
    )
    res = run_bass_kernel_spmd(
        nc, in_maps, list(range(N_CORES)), trace=trace, **trace_kw
    )
    parts = [np.asarray(r["out"]) for r in res.results]
    full = np.concatenate(parts, axis=0)  # [H, L, L]
    if full.dtype != np.float32:
        full = full.astype(np.float32)
    return full, res


def kernel(w_, offset, seq_len=None, **_ignored):
    full, _ = run(w_, offset, trace=False)
    return full

